# revision 38
# baseline (speedup 1.0000x reference)
"""Trainium2 Bass kernel for AttentionLayerWithMask (ragged prefix-mask attention).

Problem: B=1024, S=200, D=O=512.
  sqlen = mask.sum(1); query = proj_q(x[b, sqlen-1]); keys/values = x[b, :sqlen-1]
  out = tanh(attn @ V)

Algebraic rewrite (exact, up to fp reassociation):
  scores[b,s] = (Wk^T q[b]) . x[b,s]  (+ const in s -> softmax-invariant, dropped)
  out[b]      = tanh(Wv (sum_s attn[b,s] x[b,s]) + bv)

Token-packed layout: only the ~50% VALID tokens are shipped (mean sqlen ~101
of 200). Host sorts batches by length, round-robins them over the 8 cores,
cuts each batch's valid keys into <=T_PIECE-token pieces and packs pieces
into a [128 partitions x NCOL columns] grid (columns of 128 tokens each).
Each "pass" is a column range in which every partition holds tokens of ONE
batch (bid[pass, partition]).

Engine assignment (measured rates):
  VectorE : per-column fused dot  scalar_tensor_tensor(fp16 in, bf16 out,
            f32 accum) -- the accumulator taps the f32 products so fp16
            input precision is preserved (bf16 x fails the 2e-2 gate).
            The dot is VectorE's hard floor (~820ns/col, no DVE fast mode
            exists for fused two-tensor reduce), so a slice of columns is
            offloaded: product on VectorE-TT (~410ns) or GpSimd-TT, with
            the per-column sum done by ScalarE activation(accum_out).
  ScalarE : exp (+accum for z), PSUM->SBUF copies, offloaded column sums.
  GpSimd  : scatter-weight build PG[s,c,b] = onehot[s,b]*p[s,c] as one
            broadcast tensor_tensor per column-group + offloaded products.
  TensorE : ctx[b,:] += sum_s PG[s,c,b]*x[s,c,:]  accumulated in PSUM f32
            (mixed bf16 lhsT x fp16 rhs, verified on hw); per-pass query
            gather; z scatter; projections.

Sharding: pure data parallel, batch 1024 -> 8 cores x 128 partitions.
"""

import numpy as np

B, S, D, O = 1024, 200, 512, 512
NCORES = 8
P = 128                  # batches (partitions) per core
NK = D // 128            # 4 contraction chunks of 128
T_PIECE = 16             # max tokens per piece
G = 8                    # columns per DMA/compute group
N_VSTT = 3               # cols/group: fused dot on VectorE
N_VTT = 5                # cols/group: VectorE product + ScalarE sum
NEG = -1e30              # remaining cols/group: GpSimd product + ScalarE sum

_cache = {}


# ----------------------------------------------------------------------
# packing plan (host, from the actual mask)
# ----------------------------------------------------------------------

def _plan(sqlen):
    lens = sqlen.astype(np.int64) - 1                  # valid key counts >= 1
    order = np.argsort(-lens, kind="stable")           # global desc
    core_batches = [order[k::NCORES] for k in range(NCORES)]  # global idx, local order

    # pass 0 holds each batch's first piece at partition==local id, so the
    # kernel can use the un-gathered queries (onehot = identity) there.
    core_first, core_rest = [], []
    for k in range(NCORES):
        first, rest = [], []
        for lb in range(P):
            L = int(lens[core_batches[k][lb]])
            first.append((lb, 0, min(T_PIECE, L)))
            s = min(T_PIECE, L)
            while s < L:
                pl = min(T_PIECE, L - s)
                rest.append((lb, s, pl))
                s += pl
        rest.sort(key=lambda t: -t[2])
        core_first.append(first)
        core_rest.append(rest)

    npass = 1 + max((len(r) + P - 1) // P for r in core_rest)
    W = [max(pl for (_, _, pl) in f) for f in core_first]
    W = [max(W)]
    for j in range(npass - 1):
        w = 1
        for r in core_rest:
            if j * P < len(r):
                w = max(w, r[j * P][2])
        W.append(w)
    col0 = [0]
    for w in W:
        col0.append(col0[-1] + w)
    ncol = col0[-1]
    core_placed = []          # (pass, partition, lb, s, plen)
    for k in range(NCORES):
        placed = [(0, lb, lb, s, pl) for (lb, s, pl) in core_first[k]]
        placed += [(1 + i // P, i % P, lb, s, pl)
                   for i, (lb, s, pl) in enumerate(core_rest[k])]
        core_placed.append(placed)
    return core_batches, core_placed, npass, tuple(W), col0, ncol


def _groups(npass, W, col0):
    """Static group structure: per pass, list of (abs col, width<=G)."""
    out = []
    for j in range(npass):
        gs, c = [], col0[j]
        while c < col0[j] + W[j]:
            gw = min(G, col0[j] + W[j] - c)
            gs.append((c, gw))
            c += gw
        out.append(gs)
    return out


# ----------------------------------------------------------------------
# device kernel
# ----------------------------------------------------------------------

def _build_nc(npass, W, col0, ncol):
    from contextlib import ExitStack

    import concourse.bass as bass
    import concourse.tile as tile
    from concourse import bacc, mybir
    from concourse.masks import make_identity

    f32 = mybir.dt.float32
    bf16 = mybir.dt.bfloat16
    fp16 = mybir.dt.float16
    AF = mybir.ActivationFunctionType
    ALU = mybir.AluOpType

    groups = _groups(npass, W, col0)
    ncols_total = sum(W)

    nc = bacc.Bacc("TRN2", target_bir_lowering=False, debug=False, num_devices=NCORES)

    x_d = nc.dram_tensor("x", [P, ncol, D], fp16, kind="ExternalInput").ap()
    maskc_d = nc.dram_tensor("maskc", [P, ncol], f32, kind="ExternalInput").ap()
    bidc_d = nc.dram_tensor("bidc", [P, npass], f32, kind="ExternalInput").ap()
    lastT_d = nc.dram_tensor("lastT", [128, NK, P], fp16, kind="ExternalInput").ap()
    wqkT_d = nc.dram_tensor("wqkT", [128, NK, D], fp16, kind="ExternalInput").ap()
    bqk_d = nc.dram_tensor("bqk", [1, O], f32, kind="ExternalInput").ap()
    wvT_d = nc.dram_tensor("wvT", [128, NK, O], fp16, kind="ExternalInput").ap()
    bv_d = nc.dram_tensor("bv", [1, O], f32, kind="ExternalInput").ap()
    iota_d = nc.dram_tensor("iotaf", [128, 128], f32, kind="ExternalInput").ap()
    out_d = nc.dram_tensor("out", [P, O], f32, kind="ExternalOutput").ap()

    with tile.TileContext(nc) as tc:
        with ExitStack() as ctx:
            consts = ctx.enter_context(tc.tile_pool(name="consts", bufs=1))
            xg_pool = ctx.enter_context(tc.tile_pool(name="xg", bufs=7))
            prod_pool = ctx.enter_context(tc.tile_pool(name="prod", bufs=8))
            pg_pool = ctx.enter_context(tc.tile_pool(name="pg", bufs=3))
            small = ctx.enter_context(tc.tile_pool(name="small", bufs=6))
            pass_pool = ctx.enter_context(tc.tile_pool(name="pass", bufs=1))
            ps_small = ctx.enter_context(tc.tile_pool(name="psS", bufs=1, space="PSUM"))
            ps_big = ctx.enter_context(tc.tile_pool(name="psB", bufs=2, space="PSUM"))
            ps_ctx = ctx.enter_context(tc.tile_pool(name="psC", bufs=1, space="PSUM"))
            ps_prod = ctx.enter_context(tc.tile_pool(name="psP", bufs=1, space="PSUM"))
            ps_z = ctx.enter_context(tc.tile_pool(name="psZ", bufs=1, space="PSUM"))

            # ---------- constants (critical path first: qkb needs these) ----------
            wqkT_sb = consts.tile([128, NK, D], fp16, tag="wqkT")
            nc.sync.dma_start(wqkT_sb, wqkT_d)
            lastT_sb = consts.tile([128, NK, P], fp16, tag="lastT")
            nc.sync.dma_start(lastT_sb, lastT_d)
            bqk_sb = consts.tile([1, O], f32, tag="bqk")
            nc.sync.dma_start(bqk_sb, bqk_d)
            iota_sb = consts.tile([128, 128], f32, tag="iota")
            nc.sync.dma_start(iota_sb, iota_d)
            bidc_sb = consts.tile([P, npass], f32, tag="bidc")
            nc.sync.dma_start(bidc_sb, bidc_d)
            # deferred (only needed mid-loop / at the tail); DMAs issued after
            # the first x group so they don't delay the critical path
            wvT_sb = consts.tile([128, NK, O], fp16, tag="wvT")
            bv_sb = consts.tile([1, O], f32, tag="bv")
            maskc_sb = consts.tile([P, ncol], f32, tag="maskc")
            ones_sb = consts.tile([1, 128], f32, tag="ones")
            nc.vector.memset(ones_sb, 1.0)
            ident_h = consts.tile([128, 128], fp16, tag="identh")
            make_identity(nc, ident_h)
            ident_b = consts.tile([128, 128], bf16, tag="identb")
            make_identity(nc, ident_b)

            # ---------- QK[b,d] = Wqk @ last + bqk  (host-folded weights) ----------
            pqk = ps_big.tile([P, D], f32, tag="big", name="pqk")
            for kd in range(NK):
                nc.tensor.matmul(pqk, lhsT=lastT_sb[:, kd, :],
                                 rhs=wqkT_sb[:, kd, :],
                                 start=(kd == 0), stop=False)
            nc.tensor.matmul(pqk, lhsT=ones_sb, rhs=bqk_sb, start=False,
                             stop=True, skip_group_check=True)
            qkb_h = consts.tile([P, D], fp16, tag="qkb")
            nc.scalar.copy(qkb_h, pqk)

            # ---------- per-pass gather state, built upfront ----------
            ctx_ps = ps_ctx.tile([P, D], f32, tag="ctx")
            z_ps = ps_z.tile([P, 1], f32, tag="z")
            onehots, qkps = [ident_b], [qkb_h]
            for j in range(1, npass):
                onehot = pass_pool.tile([P, 128], bf16, tag=f"oh{j}")
                nc.vector.tensor_scalar(
                    out=onehot, in0=iota_sb, scalar1=bidc_sb[:, j:j + 1],
                    scalar2=None, op0=ALU.is_equal)
                ptr = ps_small.tile([128, 128], bf16, tag="trb", name=f"ohT{j}")
                nc.tensor.transpose(ptr, onehot, ident_b)
                onehotT = pass_pool.tile([128, P], bf16, tag="ohT",
                                         name=f"ohTs{j}")
                nc.scalar.copy(onehotT, ptr)
                # gather per-partition folded queries: qkp[p,:] = qkb[bid[p],:]
                qk_ps = ps_big.tile([128, D], f32, tag="big", name=f"qkg{j}")
                nc.tensor.matmul(qk_ps, lhsT=onehotT, rhs=qkb_h,
                                 start=True, stop=True)
                qkp = pass_pool.tile([128, D], fp16, tag=f"qkp{j}")
                nc.scalar.copy(qkp, qk_ps)
                onehots.append(onehot)
                qkps.append(qkp)

            # ---------- main loop: passes x column-groups ----------
            col_seen = 0
            deferred_done = [False]
            for j in range(npass):
                onehot, qkp = onehots[j], qkps[j]
                single = len(groups[j]) == 1
                if not single:
                    zpass = small.tile([P, 1], f32, tag="zp", name=f"zp{j}")
                    nc.vector.memset(zpass, 0.0)

                for (c0, gw) in groups[j]:
                    xg = xg_pool.tile([P, gw, D], fp16, tag=f"xg{gw}",
                                      name=f"xg{c0}")
                    h = min(8, gw)
                    nc.sync.dma_start(xg[:, 0:h, :], x_d[:, c0:c0 + h, :])
                    if gw > 8:
                        nc.sync.dma_start(xg[:, 8:gw, :],
                                          x_d[:, c0 + 8:c0 + gw, :])
                    if not deferred_done[0]:
                        deferred_done[0] = True
                        nc.sync.dma_start(maskc_sb, maskc_d)
                        nc.sync.dma_start(wvT_sb, wvT_d)
                        nc.sync.dma_start(bv_sb, bv_d)

                    sc = small.tile([P, gw], f32, tag=f"sc{gw}", name=f"sc{c0}")
                    n_vstt = min(N_VSTT, gw)
                    n_vtt = min(N_VTT, gw - n_vstt)
                    # offloaded columns first: their ScalarE sums overlap the
                    # VectorE fused dots that follow
                    for jj in range(n_vstt, gw):
                        prod = prod_pool.tile([P, D], fp16, tag="prodh",
                                              name=f"ph{c0}_{jj}")
                        nc.vector.tensor_tensor(out=prod, in0=xg[:, jj, :],
                                                in1=qkp, op=ALU.mult)
                        junk = prod_pool.tile([P, D], fp16, tag="junk",
                                              name=f"jk{c0}_{jj}")
                        nc.scalar.activation(junk, prod, AF.Copy,
                                             accum_out=sc[:, jj:jj + 1])
                    for jj in range(n_vstt):
                        prod = ps_prod.tile([P, D], f32, tag="pp",
                                            name=f"pr{c0}_{jj}")
                        nc.vector.scalar_tensor_tensor(
                            out=prod, in0=xg[:, jj, :], scalar=1.0, in1=qkp,
                            op0=ALU.mult, op1=ALU.mult,
                            accum_out=sc[:, jj:jj + 1])
                    nc.gpsimd.tensor_tensor(out=sc, in0=sc,
                                            in1=maskc_sb[:, c0:c0 + gw],
                                            op=ALU.add)

                    pg = small.tile([P, gw], f32, tag=f"pe{gw}", name=f"pe{c0}")
                    zg = small.tile([P, 1], f32, tag="zg", name=f"zg{c0}")
                    nc.scalar.activation(pg, sc, AF.Exp)
                    nc.vector.tensor_reduce(out=zg, in_=pg,
                                            axis=mybir.AxisListType.X,
                                            op=ALU.add)
                    if single:
                        zpass = zg
                    else:
                        nc.vector.tensor_add(zpass, zpass, zg)
                    if j == npass - 1 and (c0, gw) == groups[j][-1]:
                        zbf = small.tile([P, 1], bf16, tag=f"zbf{j}")
                        nc.vector.tensor_copy(zbf, zpass)
                        nc.tensor.matmul(z_ps, lhsT=onehots[j], rhs=zbf,
                                         start=(j == 0), stop=True,
                                         skip_group_check=True)

                    # PG[s, c, b] = onehot[s, b] * p[s, c]  (one gpsimd op)
                    PGt = pg_pool.tile([P, gw, 128], bf16, tag=f"PG{gw}",
                                       name=f"PG{c0}")
                    nc.gpsimd.tensor_tensor(
                        out=PGt,
                        in0=onehot.unsqueeze(1).to_broadcast([P, gw, 128]),
                        in1=pg.unsqueeze(2).to_broadcast([P, gw, 128]),
                        op=ALU.mult)
                    for jj in range(gw):
                        nc.tensor.matmul(
                            ctx_ps, lhsT=PGt[:, jj, :], rhs=xg[:, jj, :],
                            start=(col_seen == 0),
                            stop=(col_seen == ncols_total - 1),
                            skip_group_check=True)
                        col_seen += 1

                # scatter this pass's z by batch (last pass: emitted above)
                if j < npass - 1:
                    zbf = small.tile([P, 1], bf16, tag=f"zbf{j}")
                    nc.vector.tensor_copy(zbf, zpass)
                    nc.tensor.matmul(z_ps, lhsT=onehots[j], rhs=zbf,
                                     start=(j == 0), stop=False,
                                     skip_group_check=True)

            # ---------- normalize, project, tanh ----------
            zsb = small.tile([P, 1], f32, tag="zsb")
            nc.vector.tensor_copy(zsb, z_ps)
            rz = small.tile([P, 1], f32, tag="rz")
            nc.vector.reciprocal(rz, zsb)
            ctxT_sb = []
            for kd in range(NK):
                ctxf_k = consts.tile([P, 128], fp16, tag=f"ctxf{kd}")
                nc.scalar.activation(ctxf_k, ctx_ps[:, kd * 128:(kd + 1) * 128],
                                     AF.Copy, scale=rz)
                ptk = ps_small.tile([128, 128], fp16, tag="trh", name=f"ptk{kd}")
                nc.tensor.transpose(ptk, ctxf_k, ident_h)
                t = consts.tile([128, P], fp16, tag=f"ctxT{kd}")
                nc.scalar.copy(t, ptk)
                ctxT_sb.append(t)
            pout = ps_big.tile([P, O], f32, tag="big", name="pout")
            for kd in range(NK):
                nc.tensor.matmul(pout, lhsT=ctxT_sb[kd], rhs=wvT_sb[:, kd, :],
                                 start=(kd == 0), stop=False,
                                 skip_group_check=True)
            nc.tensor.matmul(pout, lhsT=ones_sb, rhs=bv_sb, start=False,
                             stop=True, skip_group_check=True)
            outt = consts.tile([P, O], f32, tag="outt")
            nc.scalar.activation(outt, pout, AF.Tanh)
            nc.sync.dma_start(out_d, outt)

    nc.compile()
    return nc


# ----------------------------------------------------------------------
# host prep
# ----------------------------------------------------------------------

def _host_prep(input, mask, Wq_w, Wq_b, Wk_w, Wk_b, Wv_w, Wv_b):
    fp16 = np.float16

    input = np.ascontiguousarray(input, dtype=np.float32)
    mask = np.asarray(mask)
    sqlen = mask.astype(np.int64).sum(axis=1)
    core_batches, core_pieces, npass, W, col0, ncol = _plan(sqlen)

    xh = input.astype(fp16)
    last = input[np.arange(B), sqlen - 1]              # [B, D] f32

    Wq = np.asarray(Wq_w, np.float32)
    Wk = np.asarray(Wk_w, np.float32)
    # fold the query path: qk = Wk^T(Wq last + bq) = Wqk @ last + bqk
    wqkT = np.ascontiguousarray(
        (Wq.T @ Wk).reshape(NK, 128, D).transpose(1, 0, 2)).astype(fp16)
    bqk = np.ascontiguousarray((Wk.T @ np.asarray(Wq_b, np.float32))
                               .reshape(1, O))
    wvT = np.ascontiguousarray(np.asarray(Wv_w, np.float32).T
                               .reshape(NK, 128, O).transpose(1, 0, 2)).astype(fp16)
    bv = np.ascontiguousarray(np.asarray(Wv_b, np.float32).reshape(1, O))
    iota = np.ascontiguousarray(
        np.broadcast_to(np.arange(128, dtype=np.float32)[None, :], (128, 128)))
    # Wk_b drops out of softmax (constant shift); Wv_b enters via ones-row matmul.

    in_maps = []
    for k in range(NCORES):
        gidx = core_batches[k]
        src_b = np.zeros((P, ncol), np.int64)
        src_s = np.zeros((P, ncol), np.int64)
        valid = np.zeros((P, ncol), bool)
        bidc = np.full((P, npass), -1.0, np.float32)
        for (j, p, lb, s, pl) in core_pieces[k]:
            c = col0[j]
            src_b[p, c:c + pl] = gidx[lb]
            src_s[p, c:c + pl] = np.arange(s, s + pl)
            valid[p, c:c + pl] = True
            bidc[p, j] = lb
        xcols = xh[src_b, src_s]                       # [P, ncol, D] fp16
        maskc = np.where(valid, np.float32(0.0), np.float32(NEG))
        lastT = np.ascontiguousarray(
            last[gidx].T.reshape(NK, 128, P).transpose(1, 0, 2)).astype(fp16)
        in_maps.append({
            "x": np.ascontiguousarray(xcols),
            "maskc": np.ascontiguousarray(maskc),
            "bidc": np.ascontiguousarray(bidc),
            "lastT": lastT,
            "wqkT": wqkT, "bqk": bqk, "wvT": wvT, "bv": bv,
            "iotaf": iota,
        })
    plan = (npass, W, tuple(col0), ncol, [np.asarray(g) for g in core_batches])
    return in_maps, plan


def _run(in_maps, plan, trace=False):
    from concourse.bass_utils import run_bass_kernel_spmd
    npass, W, col0, ncol, _ = plan
    key = (npass, W, ncol)
    if key not in _cache:
        _cache[key] = _build_nc(npass, list(W), list(col0), ncol)
    res = run_bass_kernel_spmd(_cache[key], in_maps, list(range(NCORES)),
                               trace=trace)
    return res


def _assemble(res, plan):
    out = np.empty((B, O), np.float32)
    core_batches = plan[4]
    for k in range(NCORES):
        out[core_batches[k]] = res.results[k]["out"]
    return out


def kernel(input, mask, Wq_w, Wq_b, Wk_w, Wk_b, Wv_w, Wv_b):
    in_maps, plan = _host_prep(input, mask, Wq_w, Wq_b, Wk_w, Wk_b, Wv_w, Wv_b)
    res = _run(in_maps, plan, trace=False)
    return _assemble(res, plan)


# revision 39
# speedup vs baseline: 1.1917x; 1.1917x over previous
"""Trainium2 Bass kernel for AttentionLayerWithMask (ragged prefix-mask attention).

Problem: B=1024, S=200, D=O=512.
  sqlen = mask.sum(1); query = proj_q(x[b, sqlen-1]); keys/values = x[b, :sqlen-1]
  out = tanh(attn @ V)

Algebraic rewrite (exact, up to fp reassociation):
  scores[b,s] = (Wk^T q[b]) . x[b,s]  (+ const in s -> softmax-invariant, dropped)
  out[b]      = tanh(Wv (sum_s attn[b,s] x[b,s]) + bv)

Token-packed layout: only the ~50% VALID tokens are shipped (mean sqlen ~101
of 200). Host sorts batches by length, round-robins them over the 8 cores,
cuts each batch's valid keys into <=T_PIECE-token pieces and packs pieces
into a [128 partitions x NCOL columns] grid (columns of 128 tokens each).
Each "pass" is a column range in which every partition holds tokens of ONE
batch (bid[pass, partition]).

Engine assignment (measured rates):
  VectorE : per-column fused dot  scalar_tensor_tensor(fp16 in, bf16 out,
            f32 accum) -- the accumulator taps the f32 products so fp16
            input precision is preserved (bf16 x fails the 2e-2 gate).
            The dot is VectorE's hard floor (~820ns/col, no DVE fast mode
            exists for fused two-tensor reduce), so a slice of columns is
            offloaded: product on VectorE-TT (~410ns) or GpSimd-TT, with
            the per-column sum done by ScalarE activation(accum_out).
  ScalarE : exp (+accum for z), PSUM->SBUF copies, offloaded column sums.
  GpSimd  : scatter-weight build PG[s,c,b] = onehot[s,b]*p[s,c] as one
            broadcast tensor_tensor per column-group + offloaded products.
  TensorE : ctx[b,:] += sum_s PG[s,c,b]*x[s,c,:]  accumulated in PSUM f32
            (mixed bf16 lhsT x fp16 rhs, verified on hw); per-pass query
            gather; z scatter; projections.

Sharding: pure data parallel, batch 1024 -> 8 cores x 128 partitions.
"""

import numpy as np

B, S, D, O = 1024, 200, 512, 512
NCORES = 8
P = 128                  # batches (partitions) per core
NK = D // 128            # 4 contraction chunks of 128
T_PIECE = 16             # max tokens per piece
G = 8                    # columns per DMA/compute group
N_VSTT = 3               # cols/group: fused dot on VectorE
N_VTT = 5                # cols/group: VectorE product + ScalarE sum
NEG = -1e30              # remaining cols/group: GpSimd product + ScalarE sum

_cache = {}


# ----------------------------------------------------------------------
# packing plan (host, from the actual mask)
# ----------------------------------------------------------------------

def _plan(sqlen):
    lens = sqlen.astype(np.int64) - 1                  # valid key counts >= 1
    order = np.argsort(-lens, kind="stable")           # global desc
    core_batches = [order[k::NCORES] for k in range(NCORES)]  # global idx, local order

    # pass 0 holds each batch's first piece at partition==local id, so the
    # kernel can use the un-gathered queries (onehot = identity) there.
    core_first, core_rest = [], []
    for k in range(NCORES):
        first, rest = [], []
        for lb in range(P):
            L = int(lens[core_batches[k][lb]])
            first.append((lb, 0, min(T_PIECE, L)))
            s = min(T_PIECE, L)
            while s < L:
                pl = min(T_PIECE, L - s)
                rest.append((lb, s, pl))
                s += pl
        rest.sort(key=lambda t: -t[2])
        core_first.append(first)
        core_rest.append(rest)

    npass = 1 + max((len(r) + P - 1) // P for r in core_rest)
    W = [max(pl for (_, _, pl) in f) for f in core_first]
    W = [max(W)]
    for j in range(npass - 1):
        w = 1
        for r in core_rest:
            if j * P < len(r):
                w = max(w, r[j * P][2])
        W.append(w)
    col0 = [0]
    for w in W:
        col0.append(col0[-1] + w)
    ncol = col0[-1]
    core_placed = []          # (pass, partition, lb, s, plen)
    for k in range(NCORES):
        placed = [(0, lb, lb, s, pl) for (lb, s, pl) in core_first[k]]
        placed += [(1 + i // P, i % P, lb, s, pl)
                   for i, (lb, s, pl) in enumerate(core_rest[k])]
        core_placed.append(placed)
    return core_batches, core_placed, npass, tuple(W), col0, ncol


def _groups(npass, W, col0):
    """Static group structure: per pass, list of (abs col, width<=G)."""
    out = []
    for j in range(npass):
        gs, c = [], col0[j]
        while c < col0[j] + W[j]:
            gw = min(G, col0[j] + W[j] - c)
            gs.append((c, gw))
            c += gw
        out.append(gs)
    return out


# ----------------------------------------------------------------------
# device kernel
# ----------------------------------------------------------------------

def _build_nc(npass, W, col0, ncol):
    from contextlib import ExitStack

    import concourse.bass as bass
    import concourse.tile as tile
    from concourse import bacc, mybir
    from concourse.masks import make_identity

    f32 = mybir.dt.float32
    bf16 = mybir.dt.bfloat16
    fp16 = mybir.dt.float16
    AF = mybir.ActivationFunctionType
    ALU = mybir.AluOpType

    groups = _groups(npass, W, col0)
    ncols_total = sum(W)

    nc = bacc.Bacc("TRN2", target_bir_lowering=False, debug=False, num_devices=NCORES)

    x_d = nc.dram_tensor("x", [P, ncol, D], fp16, kind="ExternalInput").ap()
    maskc_d = nc.dram_tensor("maskc", [P, ncol], f32, kind="ExternalInput").ap()
    bidc_d = nc.dram_tensor("bidc", [P, npass], f32, kind="ExternalInput").ap()
    lastT_d = nc.dram_tensor("lastT", [128, NK, P], fp16, kind="ExternalInput").ap()
    wqkT_d = nc.dram_tensor("wqkT", [128, NK, D], fp16, kind="ExternalInput").ap()
    bqk_d = nc.dram_tensor("bqk", [1, O], f32, kind="ExternalInput").ap()
    wvT_d = nc.dram_tensor("wvT", [128, NK, O], fp16, kind="ExternalInput").ap()
    bv_d = nc.dram_tensor("bv", [1, O], f32, kind="ExternalInput").ap()
    iota_d = nc.dram_tensor("iotaf", [128, 128], f32, kind="ExternalInput").ap()
    out_d = nc.dram_tensor("out", [P, O], f32, kind="ExternalOutput").ap()

    with tile.TileContext(nc) as tc:
        with ExitStack() as ctx:
            consts = ctx.enter_context(tc.tile_pool(name="consts", bufs=1))
            xg_pool = ctx.enter_context(tc.tile_pool(name="xg", bufs=7))
            prod_pool = ctx.enter_context(tc.tile_pool(name="prod", bufs=8))
            pg_pool = ctx.enter_context(tc.tile_pool(name="pg", bufs=2))
            small = ctx.enter_context(tc.tile_pool(name="small", bufs=4))
            pass_pool = ctx.enter_context(tc.tile_pool(name="pass", bufs=1))
            ps_small = ctx.enter_context(tc.tile_pool(name="psS", bufs=1, space="PSUM"))
            ps_big = ctx.enter_context(tc.tile_pool(name="psB", bufs=2, space="PSUM"))
            ps_ctx = ctx.enter_context(tc.tile_pool(name="psC", bufs=1, space="PSUM"))
            ps_prod = ctx.enter_context(tc.tile_pool(name="psP", bufs=1, space="PSUM"))
            ps_z = ctx.enter_context(tc.tile_pool(name="psZ", bufs=1, space="PSUM"))

            # ---------- constants (critical path first: qkb needs these) ----------
            wqkT_sb = consts.tile([128, NK, D], fp16, tag="wqkT")
            nc.sync.dma_start(wqkT_sb, wqkT_d)
            lastT_sb = consts.tile([128, NK, P], fp16, tag="lastT")
            nc.sync.dma_start(lastT_sb, lastT_d)
            bqk_sb = consts.tile([1, O], f32, tag="bqk")
            nc.sync.dma_start(bqk_sb, bqk_d)
            iota_sb = consts.tile([128, 128], f32, tag="iota")
            nc.sync.dma_start(iota_sb, iota_d)
            bidc_sb = consts.tile([P, npass], f32, tag="bidc")
            nc.sync.dma_start(bidc_sb, bidc_d)
            # deferred (only needed mid-loop / at the tail); DMAs issued after
            # the first x group so they don't delay the critical path
            wvT_sb = consts.tile([128, NK, O], fp16, tag="wvT")
            bv_sb = consts.tile([1, O], f32, tag="bv")
            maskc_sb = consts.tile([P, ncol], f32, tag="maskc")
            ones_sb = consts.tile([1, 128], f32, tag="ones")
            nc.vector.memset(ones_sb, 1.0)
            ident_h = consts.tile([128, 128], fp16, tag="identh")
            make_identity(nc, ident_h)
            ident_b = consts.tile([128, 128], bf16, tag="identb")
            make_identity(nc, ident_b)

            # ---------- QK[b,d] = Wqk @ last + bqk  (host-folded weights) ----------
            pqk = ps_big.tile([P, D], f32, tag="big", name="pqk")
            for kd in range(NK):
                nc.tensor.matmul(pqk, lhsT=lastT_sb[:, kd, :],
                                 rhs=wqkT_sb[:, kd, :],
                                 start=(kd == 0), stop=False)
            nc.tensor.matmul(pqk, lhsT=ones_sb, rhs=bqk_sb, start=False,
                             stop=True, skip_group_check=True)
            qkb_h = consts.tile([P, D], fp16, tag="qkb")
            nc.scalar.copy(qkb_h, pqk)

            # ---------- per-pass gather state, built upfront ----------
            ctx_ps = ps_ctx.tile([P, D], f32, tag="ctx")
            z_ps = ps_z.tile([P, 1], f32, tag="z")
            onehots, qkps = [ident_b], [qkb_h]
            for j in range(1, npass):
                onehot = pass_pool.tile([P, 128], bf16, tag=f"oh{j}")
                nc.vector.tensor_scalar(
                    out=onehot, in0=iota_sb, scalar1=bidc_sb[:, j:j + 1],
                    scalar2=None, op0=ALU.is_equal)
                ptr = ps_small.tile([128, 128], bf16, tag="trb", name=f"ohT{j}")
                nc.tensor.transpose(ptr, onehot, ident_b)
                onehotT = pass_pool.tile([128, P], bf16, tag="ohT",
                                         name=f"ohTs{j}")
                nc.scalar.copy(onehotT, ptr)
                # gather per-partition folded queries: qkp[p,:] = qkb[bid[p],:]
                qk_ps = ps_big.tile([128, D], f32, tag="big", name=f"qkg{j}")
                nc.tensor.matmul(qk_ps, lhsT=onehotT, rhs=qkb_h,
                                 start=True, stop=True)
                qkp = pass_pool.tile([128, D], fp16, tag=f"qkp{j}")
                nc.scalar.copy(qkp, qk_ps)
                onehots.append(onehot)
                qkps.append(qkp)

            # ---------- main loop: passes x column-groups ----------
            col_seen = 0
            deferred_done = [False]
            for j in range(npass):
                onehot, qkp = onehots[j], qkps[j]
                single = len(groups[j]) == 1
                if not single:
                    zpass = small.tile([P, 1], f32, tag="zp", name=f"zp{j}")
                    nc.vector.memset(zpass, 0.0)

                for (c0, gw) in groups[j]:
                    xg = xg_pool.tile([P, gw, D], fp16, tag=f"xg{gw}",
                                      name=f"xg{c0}")
                    h = min(8, gw)
                    nc.sync.dma_start(xg[:, 0:h, :], x_d[:, c0:c0 + h, :])
                    if gw > 8:
                        nc.sync.dma_start(xg[:, 8:gw, :],
                                          x_d[:, c0 + 8:c0 + gw, :])
                    if not deferred_done[0]:
                        deferred_done[0] = True
                        nc.sync.dma_start(maskc_sb, maskc_d)
                        nc.sync.dma_start(wvT_sb, wvT_d)
                        nc.sync.dma_start(bv_sb, bv_d)

                    sc = small.tile([P, gw], f32, tag=f"sc{gw}", name=f"sc{c0}")
                    n_vstt = min(N_VSTT, gw)
                    n_vtt = min(N_VTT, gw - n_vstt)
                    # offloaded columns first: their ScalarE sums overlap the
                    # VectorE fused dots that follow
                    for jj in range(n_vstt, gw):
                        prod = prod_pool.tile([P, D], fp16, tag="prodh",
                                              name=f"ph{c0}_{jj}")
                        nc.vector.tensor_tensor(out=prod, in0=xg[:, jj, :],
                                                in1=qkp, op=ALU.mult)
                        junk = prod_pool.tile([P, D], fp16, tag="junk",
                                              name=f"jk{c0}_{jj}")
                        nc.scalar.activation(junk, prod, AF.Copy,
                                             accum_out=sc[:, jj:jj + 1])
                    for jj in range(n_vstt):
                        prod = ps_prod.tile([P, D], f32, tag="pp",
                                            name=f"pr{c0}_{jj}")
                        nc.vector.scalar_tensor_tensor(
                            out=prod, in0=xg[:, jj, :], scalar=1.0, in1=qkp,
                            op0=ALU.mult, op1=ALU.mult,
                            accum_out=sc[:, jj:jj + 1])
                    nc.gpsimd.tensor_tensor(out=sc, in0=sc,
                                            in1=maskc_sb[:, c0:c0 + gw],
                                            op=ALU.add)

                    pg = small.tile([P, gw], f32, tag=f"pe{gw}", name=f"pe{c0}")
                    zg = small.tile([P, 1], f32, tag="zg", name=f"zg{c0}")
                    nc.scalar.activation(pg, sc, AF.Exp)
                    nc.vector.tensor_reduce(out=zg, in_=pg,
                                            axis=mybir.AxisListType.X,
                                            op=ALU.add)
                    if single:
                        zpass = zg
                    else:
                        nc.vector.tensor_add(zpass, zpass, zg)
                    if j == npass - 1 and (c0, gw) == groups[j][-1]:
                        zbf = small.tile([P, 1], bf16, tag=f"zbf{j}")
                        nc.vector.tensor_copy(zbf, zpass)
                        nc.tensor.matmul(z_ps, lhsT=onehots[j], rhs=zbf,
                                         start=(j == 0), stop=True,
                                         skip_group_check=True)

                    # PG[s, c, b] = onehot[s, b] * p[s, c]  (one gpsimd op)
                    PGt = pg_pool.tile([P, gw, 128], bf16, tag=f"PG{gw}",
                                       name=f"PG{c0}")
                    nc.gpsimd.tensor_tensor(
                        out=PGt,
                        in0=onehot.unsqueeze(1).to_broadcast([P, gw, 128]),
                        in1=pg.unsqueeze(2).to_broadcast([P, gw, 128]),
                        op=ALU.mult)
                    for jj in range(gw):
                        nc.tensor.matmul(
                            ctx_ps, lhsT=PGt[:, jj, :], rhs=xg[:, jj, :],
                            start=(col_seen == 0),
                            stop=(col_seen == ncols_total - 1),
                            skip_group_check=True)
                        col_seen += 1

                # scatter this pass's z by batch (last pass: emitted above)
                if j < npass - 1:
                    zbf = small.tile([P, 1], bf16, tag=f"zbf{j}")
                    nc.vector.tensor_copy(zbf, zpass)
                    nc.tensor.matmul(z_ps, lhsT=onehots[j], rhs=zbf,
                                     start=(j == 0), stop=False,
                                     skip_group_check=True)

            # ---------- normalize, project, tanh ----------
            zsb = small.tile([P, 1], f32, tag="zsb")
            nc.vector.tensor_copy(zsb, z_ps)
            rz = small.tile([P, 1], f32, tag="rz")
            nc.vector.reciprocal(rz, zsb)
            ctxT_sb = []
            for kd in range(NK):
                ctxf_k = consts.tile([P, 128], fp16, tag=f"ctxf{kd}")
                nc.scalar.activation(ctxf_k, ctx_ps[:, kd * 128:(kd + 1) * 128],
                                     AF.Copy, scale=rz)
                ptk = ps_small.tile([128, 128], fp16, tag="trh", name=f"ptk{kd}")
                nc.tensor.transpose(ptk, ctxf_k, ident_h)
                t = consts.tile([128, P], fp16, tag=f"ctxT{kd}")
                nc.scalar.copy(t, ptk)
                ctxT_sb.append(t)
            pout = ps_big.tile([P, O], f32, tag="big", name="pout")
            for kd in range(NK):
                nc.tensor.matmul(pout, lhsT=ctxT_sb[kd], rhs=wvT_sb[:, kd, :],
                                 start=(kd == 0), stop=False,
                                 skip_group_check=True)
            nc.tensor.matmul(pout, lhsT=ones_sb, rhs=bv_sb, start=False,
                             stop=True, skip_group_check=True)
            outt = consts.tile([P, O], f32, tag="outt")
            nc.scalar.activation(outt, pout, AF.Tanh)
            nc.sync.dma_start(out_d, outt)

    nc.compile()
    return nc


# ----------------------------------------------------------------------
# host prep
# ----------------------------------------------------------------------

def _host_prep(input, mask, Wq_w, Wq_b, Wk_w, Wk_b, Wv_w, Wv_b):
    fp16 = np.float16

    input = np.ascontiguousarray(input, dtype=np.float32)
    mask = np.asarray(mask)
    sqlen = mask.astype(np.int64).sum(axis=1)
    core_batches, core_pieces, npass, W, col0, ncol = _plan(sqlen)

    xh = input.astype(fp16)
    last = input[np.arange(B), sqlen - 1]              # [B, D] f32

    Wq = np.asarray(Wq_w, np.float32)
    Wk = np.asarray(Wk_w, np.float32)
    # fold the query path: qk = Wk^T(Wq last + bq) = Wqk @ last + bqk
    wqkT = np.ascontiguousarray(
        (Wq.T @ Wk).reshape(NK, 128, D).transpose(1, 0, 2)).astype(fp16)
    bqk = np.ascontiguousarray((Wk.T @ np.asarray(Wq_b, np.float32))
                               .reshape(1, O))
    wvT = np.ascontiguousarray(np.asarray(Wv_w, np.float32).T
                               .reshape(NK, 128, O).transpose(1, 0, 2)).astype(fp16)
    bv = np.ascontiguousarray(np.asarray(Wv_b, np.float32).reshape(1, O))
    iota = np.ascontiguousarray(
        np.broadcast_to(np.arange(128, dtype=np.float32)[None, :], (128, 128)))
    # Wk_b drops out of softmax (constant shift); Wv_b enters via ones-row matmul.

    in_maps = []
    for k in range(NCORES):
        gidx = core_batches[k]
        src_b = np.zeros((P, ncol), np.int64)
        src_s = np.zeros((P, ncol), np.int64)
        valid = np.zeros((P, ncol), bool)
        bidc = np.full((P, npass), -1.0, np.float32)
        for (j, p, lb, s, pl) in core_pieces[k]:
            c = col0[j]
            src_b[p, c:c + pl] = gidx[lb]
            src_s[p, c:c + pl] = np.arange(s, s + pl)
            valid[p, c:c + pl] = True
            bidc[p, j] = lb
        xcols = xh[src_b, src_s]                       # [P, ncol, D] fp16
        maskc = np.where(valid, np.float32(0.0), np.float32(NEG))
        lastT = np.ascontiguousarray(
            last[gidx].T.reshape(NK, 128, P).transpose(1, 0, 2)).astype(fp16)
        in_maps.append({
            "x": np.ascontiguousarray(xcols),
            "maskc": np.ascontiguousarray(maskc),
            "bidc": np.ascontiguousarray(bidc),
            "lastT": lastT,
            "wqkT": wqkT, "bqk": bqk, "wvT": wvT, "bv": bv,
            "iotaf": iota,
        })
    plan = (npass, W, tuple(col0), ncol, [np.asarray(g) for g in core_batches])
    return in_maps, plan


def _run(in_maps, plan, trace=False):
    from concourse.bass_utils import run_bass_kernel_spmd
    npass, W, col0, ncol, _ = plan
    key = (npass, W, ncol)
    if key not in _cache:
        _cache[key] = _build_nc(npass, list(W), list(col0), ncol)
    res = run_bass_kernel_spmd(_cache[key], in_maps, list(range(NCORES)),
                               trace=trace)
    return res


def _assemble(res, plan):
    out = np.empty((B, O), np.float32)
    core_batches = plan[4]
    for k in range(NCORES):
        out[core_batches[k]] = res.results[k]["out"]
    return out


def kernel(input, mask, Wq_w, Wq_b, Wk_w, Wk_b, Wv_w, Wv_b):
    in_maps, plan = _host_prep(input, mask, Wq_w, Wq_b, Wk_w, Wk_b, Wv_w, Wv_b)
    res = _run(in_maps, plan, trace=False)
    return _assemble(res, plan)
